# revision 4
# baseline (speedup 1.0000x reference)
"""Trainium2 Bass kernel for nn_AsymmetricProjectedLinear (8 NeuronCores).

Reference computes out = x @ W_large^T with
    W_large = (A_out @ B_out) @ W_small @ (A_in @ B_in)^T    [4096, 4096]

W_large is never materialized. Factored:
    G  = B_out @ W_small                        [64, 1024]
    MT = G @ B_in^T                             [64, 64]    (= M^T)
    C  = M @ A_out^T                            [64, 4096]
    out = (x @ A_in) @ C                        [4096t, 4096]

Sharding: tokens (B*S = 4096) split 512/core across 8 cores; weights
replicated (a 16KB AllReduce for M costs ~50us wall on this runtime, so
every core redundantly computes M from the full W_small). Host work is
layout-only (transpose/pack/slice/dtype-cast); all FLOPs on-device.

Key structure vs the previous version (59us -> target ~38us):
  - C is precomputed once in prework instead of applying M per block
    (old stage2), so each block's out tiles depend only on u1(b) and C.
    The old per-block gpsimd SBUF dup of t2 (which landed late in the
    schedule and serialized the tail) is gone entirely.
  - Partition-half duplication is done by the PE itself via matmul
    tile_position: MT and C are computed into BOTH psum partition
    halves (col tile offset 64), and stage1 writes the two token
    halves of u1T directly to partitions 0-63 / 64-127. stage5 then
    runs dual-pumped K=64 matmul pairs (row tile offsets 0/64), which
    the PE executes concurrently.
  - Out tiles are [128, 2048] (524KB DMAs) with enough bufs that no
    stage ever waits on an out transfer completing.
  - Wire order: small weights + W_small first (gates C), then x in
    1MB pieces per ring; out DMAs ride behind in-stream on both HWDGE
    rings. Wire total ~11.8MB at ~400GB/s ~= 30us + ~7us fixed
    preamble.
"""

import numpy as np

import concourse.bass as bass
import concourse.mybir as mybir
import concourse.tile as tile
from concourse import bacc
from concourse.bass_utils import run_bass_kernel_spmd

N_CORES = 8
Bsz, S, D = 2, 2048, 4096
TOK = Bsz * S          # 4096 tokens
T = TOK // N_CORES     # 512 tokens per core
TB = 256               # tokens per pipeline block
NBLK = T // TB         # 2 blocks
RANK = 64
DS = 1024              # d_small

F32 = mybir.dt.float32
BF16 = mybir.dt.bfloat16

_nc_cache = {}


def build():
    if "nc" in _nc_cache:
        return _nc_cache["nc"]
    nc = bacc.Bacc("TRN2", target_bir_lowering=False, debug=False,
                   num_devices=N_CORES)

    # x_p: per block, 32 d-tiles of [128, TB] packed -> [128, 32*TB]
    x_p = nc.dram_tensor("x_p", [NBLK, 128, 32 * TB], BF16, kind="ExternalInput")
    # b_outT | b_inT | a_in packed -> single front DMA
    wsm_p = nc.dram_tensor("wsm_p", [128, 8 * RANK * 2 + 32 * RANK], BF16,
                           kind="ExternalInput")
    # A_out^T stacked: parts 0-63 = cols 0:2048, parts 64-127 = cols 2048:4096
    a_out2 = nc.dram_tensor("a_out2", [128, 2048], BF16, kind="ExternalInput")
    w_p = nc.dram_tensor("w_p", [128, 8 * DS], BF16, kind="ExternalInput")
    ident = nc.dram_tensor("ident", [RANK, RANK], BF16, kind="ExternalInput")
    out = nc.dram_tensor("out", [T, D], BF16, kind="ExternalOutput")

    with tile.TileContext(nc) as tc:
        with (
            tc.tile_pool(name="const", bufs=1) as cpool,
            tc.tile_pool(name="xin", bufs=2) as xpool,
            tc.tile_pool(name="outp", bufs=4) as opool,
            tc.tile_pool(name="interm", bufs=2) as ipool,
            tc.tile_pool(name="ps_pre", bufs=2, space="PSUM") as ps_pre,
            tc.tile_pool(name="ps_u1", bufs=2, space="PSUM") as ps_u1,
            tc.tile_pool(name="ps_o", bufs=4, space="PSUM") as ps_o,
        ):
            # ---- input streams, interleaved across BOTH HWDGE rings ----
            # Each ring drains FIFO, so byte position = arrival time.
            # W_small + small weights land first (they gate C); x follows
            # in 1MB pieces; out DMAs are appended behind as blocks drain.
            wsm_s = cpool.tile([128, 8 * RANK * 2 + 32 * RANK], BF16)
            b_outT_s = wsm_s[:, 0:8 * RANK]
            b_inT_s = wsm_s[:, 8 * RANK:16 * RANK]
            a_in_s = wsm_s[:, 16 * RANK:]
            ident_s = cpool.tile([RANK, RANK], BF16)
            a_out_s = cpool.tile([128, 2048], BF16)
            w_tiles = [None] * 8
            x_tiles = [[None] * 2 for _ in range(NBLK)]

            nc.sync.dma_start(out=wsm_s[:, :], in_=wsm_p.ap())
            nc.scalar.dma_start(out=ident_s[:, :], in_=ident.ap())
            for j in range(8):
                wt = cpool.tile([128, DS], BF16, tag=f"w{j}")
                eng = nc.sync if j % 2 == 0 else nc.scalar
                eng.dma_start(out=wt[:, :], in_=w_p.ap()[:, j * DS:(j + 1) * DS])
                w_tiles[j] = wt
            nc.scalar.dma_start(out=a_out_s[:, :], in_=a_out2.ap())
            for b in range(NBLK):
                for p in range(2):      # 16 d-tiles = 1.05MB per piece
                    xt = xpool.tile([128, 16 * TB], BF16, tag=f"x{p}")
                    eng = nc.sync if p == 0 else nc.scalar
                    eng.dma_start(
                        out=xt[:, :],
                        in_=x_p.ap()[b, :, p * 16 * TB:(p + 1) * 16 * TB],
                    )
                    x_tiles[b][p] = xt

            # ---- prework: G -> G^T -> MT (both halves) -> C (both halves)
            g_ps = [ps_pre.tile([RANK, 512], F32, tag="pre", name=f"g_ps{h}")
                    for h in range(2)]
            for j in range(8):
                for h in range(2):
                    nc.tensor.matmul(
                        g_ps[h][:, :],
                        b_outT_s[:, j * RANK:(j + 1) * RANK],
                        w_tiles[j][:, h * 512:(h + 1) * 512],
                        start=(j == 0), stop=(j == 7),
                    )
            g_s = cpool.tile([RANK, DS], BF16)
            nc.vector.tensor_copy(g_s[:, 0:512], g_ps[0][:, :])
            nc.scalar.copy(g_s[:, 512:1024], g_ps[1][:, :])

            gT_s = cpool.tile([128, 8 * RANK], BF16)
            for it in range(8):
                gt_ps = ps_pre.tile([128, RANK], BF16, tag="pre")
                nc.tensor.transpose(
                    gt_ps[:, :], g_s[:, it * 128:(it + 1) * 128], ident_s[:, :])
                eng = nc.vector if it % 2 == 0 else nc.scalar
                if it % 2 == 0:
                    eng.tensor_copy(gT_s[:, it * RANK:(it + 1) * RANK], gt_ps[:, :])
                else:
                    eng.copy(gT_s[:, it * RANK:(it + 1) * RANK], gt_ps[:, :])

            # MT = G @ B_in^T, written to both partition halves so C's
            # chunk 4-7 matmuls (operands on parts 64-127) can read it.
            mt_ps = ps_pre.tile([128, RANK], F32, tag="pre")
            for ch in range(2):
                for it in range(8):
                    nc.tensor.matmul(
                        mt_ps[ch * RANK:(ch + 1) * RANK, :],
                        gT_s[:, it * RANK:(it + 1) * RANK],
                        b_inT_s[:, it * RANK:(it + 1) * RANK],
                        start=(it == 0), stop=(it == 7),
                    )
            mt_s = cpool.tile([128, RANK], BF16)
            nc.vector.tensor_copy(mt_s[:, :], mt_ps[:, :])

            # C = M @ A_out^T = MT^T @ A_out^T, chunk n covers out cols
            # n*512:(n+1)*512; written to both partition halves.
            c_s = cpool.tile([128, D], BF16)
            for n in range(8):
                c_ps = ps_pre.tile([128, 512], F32, tag="pre")
                h2 = n // 4
                col = (n % 4) * 512
                for ch in range(2):
                    nc.tensor.matmul(
                        c_ps[ch * RANK:(ch + 1) * RANK, :],
                        mt_s[h2 * RANK:(h2 + 1) * RANK, :],
                        a_out_s[h2 * RANK:(h2 + 1) * RANK, col:col + 512],
                        start=True, stop=True,
                    )
                if n % 2 == 0:
                    nc.vector.tensor_copy(c_s[:, n * 512:(n + 1) * 512], c_ps[:, :])
                else:
                    nc.scalar.copy(c_s[:, n * 512:(n + 1) * 512], c_ps[:, :])

            # ---- per token block: u1T then out = u1 @ C ----
            for b in range(NBLK):
                u1_ps = ps_u1.tile([128, 128], F32, tag="u1")
                for h in range(2):      # token half -> psum partition half
                    for m in range(32):
                        xt = x_tiles[b][m // 16]
                        col = (m % 16) * TB + h * 128
                        nc.tensor.matmul(
                            u1_ps[h * RANK:(h + 1) * RANK, :],
                            a_in_s[:, m * RANK:(m + 1) * RANK],
                            xt[:, col:col + 128],
                            start=(m == 0), stop=(m == 31),
                        )
                u1d = ipool.tile([128, 128], BF16, tag="u1d")
                nc.vector.tensor_copy(u1d[:, :], u1_ps[:, :])

                r0 = b * TB
                for cg in range(2):     # out column group of 2048
                    o_t0 = opool.tile([128, 2048], BF16, tag="o0")
                    o_t1 = opool.tile([128, 2048], BF16, tag="o1")
                    for k in range(4):
                        n = cg * 4 + k
                        po0 = ps_o.tile([128, 512], F32, tag="po")
                        po1 = ps_o.tile([128, 512], F32, tag="po")
                        nc.tensor.matmul(
                            po0[:, :], u1d[0:RANK, :],
                            c_s[0:RANK, n * 512:(n + 1) * 512],
                            start=True, stop=True,
                        )
                        nc.tensor.matmul(
                            po1[:, :], u1d[RANK:128, :],
                            c_s[RANK:128, n * 512:(n + 1) * 512],
                            start=True, stop=True,
                        )
                        nc.vector.tensor_copy(o_t0[:, k * 512:(k + 1) * 512], po0[:, :])
                        nc.scalar.copy(o_t1[:, k * 512:(k + 1) * 512], po1[:, :])
                    e0 = nc.sync if cg == 0 else nc.scalar
                    e1 = nc.scalar if cg == 0 else nc.sync
                    e0.dma_start(
                        out=out.ap()[r0:r0 + 128, cg * 2048:(cg + 1) * 2048],
                        in_=o_t0[:, :])
                    e1.dma_start(
                        out=out.ap()[r0 + 128:r0 + 256, cg * 2048:(cg + 1) * 2048],
                        in_=o_t1[:, :])

    nc.compile()
    _nc_cache["nc"] = nc
    return nc


def _prep_in_maps(x, W_small, A_out, B_out, A_in, B_in):
    import ml_dtypes
    f = ml_dtypes.bfloat16
    x2 = np.asarray(x, dtype=f).reshape(TOK, D)
    a_in_p = np.ascontiguousarray(
        np.asarray(A_in, f).reshape(32, 128, RANK).transpose(1, 0, 2)
    ).reshape(128, 32 * RANK)
    a_outT = np.asarray(A_out, f).T            # [64, 4096]
    a_out2 = np.ascontiguousarray(
        np.concatenate([a_outT[:, :2048], a_outT[:, 2048:]], axis=0))
    b_inT_p = np.ascontiguousarray(
        np.asarray(B_in, f).T.reshape(8, 128, RANK).transpose(1, 0, 2)
    ).reshape(128, 8 * RANK)
    b_outT_p = np.ascontiguousarray(
        np.asarray(B_out, f).T.reshape(8, 128, RANK).transpose(1, 0, 2)
    ).reshape(128, 8 * RANK)
    wsm_p = np.ascontiguousarray(
        np.concatenate([b_outT_p, b_inT_p, a_in_p], axis=1))
    w_p = np.ascontiguousarray(
        np.asarray(W_small, f).reshape(8, 128, DS).transpose(1, 0, 2)
    ).reshape(128, 8 * DS)
    ident = np.eye(RANK, dtype=f)

    shared = {
        "wsm_p": wsm_p, "a_out2": a_out2, "w_p": w_p, "ident": ident,
    }
    in_maps = []
    for c in range(N_CORES):
        xs = x2[c * T:(c + 1) * T, :]            # [T, 4096]
        xp = np.ascontiguousarray(
            xs.T                                  # [4096, T]
            .reshape(32, 128, NBLK, TB)           # d-tile, p, blk, t
            .transpose(2, 1, 0, 3)                # blk, p, d-tile, t
        ).reshape(NBLK, 128, 32 * TB)
        in_maps.append({"x_p": xp, **shared})
    return in_maps


def _run(inputs, trace=False):
    nc = build()
    in_maps = _prep_in_maps(**inputs)
    res = run_bass_kernel_spmd(
        nc, in_maps, core_ids=list(range(N_CORES)), trace=trace
    )
    out = np.concatenate(
        [np.asarray(res.results[c]["out"], dtype=np.float32)
         for c in range(N_CORES)], axis=0
    ).reshape(Bsz, S, D)
    return out, res


def kernel(**inputs) -> np.ndarray:
    out, _ = _run(inputs, trace=False)
    return out
